# revision 27
# baseline (speedup 1.0000x reference)
"""Binarized conv2d kernel for Trainium2, SPMD over 8 NeuronCores.

Math (forward-value equivalent of the reference):
    real_w  = sum_k RV[k] * weights[k]          # [256,256,3,3], exact fp32 on DVE
    scale   = mean(|real_w|, axis=(1,2,3))      # per out-channel
    out     = conv2d(sign(x), sign(real_w), pad=1) * (scale * alpha)

sign(x) and sign(real_w) are {-1,0,+1} which are exact in fp8e4, so the conv
is computed with fp8 DoubleRow matmuls (exact integer accumulation in fp32
PSUM) and the per-channel scale*alpha is applied on PSUM evacuation.

Sharding: data-parallel over batch, 4 images per core; weights/RV/alpha
replicated. No collectives.
"""

import numpy as np
from contextlib import ExitStack

import concourse.bass as bass
import concourse.bacc as bacc
import concourse.tile as tile
from concourse import mybir
from concourse.bass_utils import run_bass_kernel_spmd
from concourse.masks import make_identity

# Problem shapes (hardcoded per contract)
B, C, H, W = 32, 256, 56, 56
K, KS = 4, 3
NCORES = 8
BL = B // NCORES            # images per core

PW = W + 2                  # padded width 58
PLANE = PW * PW             # 3364
PL = 3376                   # plane stride (>= 1+PLANE+1, multiple of 16)
GO = 1                      # guard offset: plane data starts at elem 1
RPC = 8                     # rows per chunk
CHUNK = RPC * PW            # 464 elems per matmul (one PSUM bank)
NCHUNK = H // RPC           # 7 chunks: psum tile A gets 4, tile B gets 3
PT_CHUNKS = (4, 3)
CIH = C // 128              # 2 ci halves
COH = C // 128              # 2 co halves
TAPS = KS * KS              # 9

F32 = mybir.dt.float32
FP8 = mybir.dt.float8e4
BF16 = mybir.dt.bfloat16

USE_DR = True               # fp8 DoubleRow (2x matmul) vs bf16

_cache = {}


def _build():
    act_dt = FP8 if USE_DR else BF16
    nc = bacc.Bacc("TRN2", target_bir_lowering=False, debug=False,
                   num_devices=NCORES)
    x_d = nc.dram_tensor("x", [BL, C, H, W], F32, kind="ExternalInput")
    w_d = nc.dram_tensor("weights", [K, C, C, KS, KS], F32, kind="ExternalInput")
    rv_d = nc.dram_tensor("RV", [K + 1], F32, kind="ExternalInput")
    al_d = nc.dram_tensor("alpha", [C, 1, 1], F32, kind="ExternalInput")
    o_d = nc.dram_tensor("out", [BL, C, H, W], F32, kind="ExternalOutput")

    with tile.TileContext(nc) as tc, ExitStack() as ctx:
        consts = ctx.enter_context(tc.tile_pool(name="consts", bufs=1))
        wstage = ctx.enter_context(tc.tile_pool(name="wstage", bufs=16))
        wwork = ctx.enter_context(tc.tile_pool(name="wwork", bufs=2))
        xin = ctx.enter_context(tc.tile_pool(name="xin", bufs=2))
        xpads = ctx.enter_context(tc.tile_pool(name="xpads", bufs=1))
        outp = ctx.enter_context(tc.tile_pool(name="outp", bufs=2))

        # --- tiny constant loads on the ACT HWDGE ring (keeps the sync
        # ring free for the big weight DMAs) -------------------------------
        rv = consts.tile([128, K], F32, tag="rv")
        rv_src = bass.AP(tensor=rv_d.ap().tensor, offset=0,
                         ap=[[0, 128], [1, K]])
        nc.scalar.dma_start(out=rv, in_=rv_src)
        alpha_sb = []
        for h in range(COH):
            t = consts.tile([128, 1], F32, tag=f"alpha{h}")
            nc.scalar.dma_start(out=t,
                                in_=al_d.ap()[h * 128:(h + 1) * 128, 0, :])
            alpha_sb.append(t)

        # Padded planes: zero only the pad borders on DVE (tiny strided
        # memsets — the interior is fully overwritten by sign(x) each image
        # and pads are never written again). GpSimd keeps only the identity.
        xpad = []
        for i in range(2):
            t = xpads.tile([128, CIH, PL], act_dt, tag=f"xpad{i}",
                           name=f"xpad{i}")
            for s in range(CIH):
                pl = t[:, s, :]
                # guard + top row + (1,0)
                nc.vector.memset(pl[:, 0:GO + PW + 1], 0.0)
                # (y,0) and (y,57) for y=1..56
                nc.vector.memset(
                    pl[:, GO + PW:GO + PW + H * PW].rearrange(
                        "p (r c) -> p r c", c=PW)[:, :, 0:1], 0.0)
                nc.vector.memset(
                    pl[:, GO + PW + PW - 1:GO + PW + PW - 1 + H * PW].rearrange(
                        "p (r c) -> p r c", c=PW)[:, :, 0:1], 0.0)
                # bottom row + trailing guard/pad
                nc.vector.memset(pl[:, GO + (PW - 1) * PW:PL], 0.0)
            xpad.append(t)
        ident = consts.tile([128, 128], act_dt, tag="ident")
        make_identity(nc, ident)

        wT = consts.tile([128, TAPS, COH, CIH, 128], act_dt, tag="wT")
        scale_alpha = [consts.tile([128, 1], F32, tag=f"sa{h}", name=f"sa{h}")
                       for h in range(COH)]

        # --- weight prep for one co-half: DMA, mix, scale, sign ------------
        # ci-split (HCI columns at a time) so the mix/sign tail trails the
        # weight DMA by one sub-pass instead of the whole 4.7MB.
        HCI = C // CIH * TAPS  # 1152 columns per ci-half
        def prep_half(h):
            # (TensorScalarPtr is DVE-only in walrus codegen — Pool rejects)
            mixeng = nc.vector
            wmix = wwork.tile([128, C * TAPS], F32, tag="wmix", name="wmix")
            ws = wwork.tile([128, C * TAPS], act_dt, tag=f"wsign{h}", bufs=1,
                            name=f"wsign{h}")
            for ci in range(CIH):
                for k in range(K):
                    wk = wstage.tile([128, HCI], F32, tag="wsb", name="wk")
                    nc.sync.dma_start(
                        out=wk,
                        in_=w_d.ap()[k, h * 128:(h + 1) * 128,
                                     ci * (C // CIH):(ci + 1) * (C // CIH)]
                        .rearrange("p c a b -> p (c a b)"))
                    dst = wmix[:, ci * HCI:(ci + 1) * HCI]
                    mixeng.scalar_tensor_tensor(
                        dst, wk, rv[:, k:k + 1], wk if k == 0 else dst,
                        mybir.AluOpType.mult,
                        mybir.AluOpType.bypass if k == 0 else
                        mybir.AluOpType.add)
                nc.scalar.sign(ws[:, ci * HCI:(ci + 1) * HCI],
                               wmix[:, ci * HCI:(ci + 1) * HCI])
            return ws, wmix

        # |real_w| row-sums on ACT (Abs + accumulate), tiny combines on DVE
        def reduce_half(h, wmix):
            parts = []
            for ci in range(CIH):
                trash = wwork.tile([128, HCI], F32, tag="trash", bufs=1,
                                   name="trash")
                ab = consts.tile([128, 1], F32, tag=f"ab{h}{ci}",
                                 name=f"ab{h}{ci}")
                nc.scalar.activation(trash, wmix[:, ci * HCI:(ci + 1) * HCI],
                                     mybir.ActivationFunctionType.Abs,
                                     accum_out=ab)
                parts.append(ab)
            tmp = consts.tile([128, 1], F32, tag=f"abt{h}", name=f"abt{h}")
            nc.vector.scalar_tensor_tensor(
                tmp, parts[0], 1.0, parts[1],
                mybir.AluOpType.bypass, mybir.AluOpType.add)
            nc.vector.scalar_tensor_tensor(
                scale_alpha[h], tmp, 1.0 / (C * TAPS), alpha_sb[h],
                mybir.AluOpType.mult, mybir.AluOpType.mult)

        # --- transpose one co-half's sign-weights into wT -------------------
        # Groups of 4 [128,128] transposes fill the dedicated 1-bank psum
        # tile (tag tps); each group is evacuated with one DVE copy.
        def transpose_half(h, wsgn, cpsum):
            wsv = wsgn.rearrange("p (ci t) -> p ci t", t=TAPS)
            pairs = [(t, ci) for t in range(TAPS) for ci in range(CIH)]
            for g0 in range(0, len(pairs), 4):
                grp = pairs[g0:g0 + 4]
                tp = cpsum.tile([128, 512], F32, tag="tps", bufs=1, name="tp")
                for i, (tap, ci) in enumerate(grp):
                    nc.tensor.matmul(
                        tp[:, i * 128:(i + 1) * 128],
                        wsv[:, ci * 128:(ci + 1) * 128, tap], ident,
                        start=True, stop=True)
                t0, ci0 = grp[0]
                ntap = len(grp) // CIH
                nc.scalar.copy(
                    wT[:, t0:t0 + ntap, h, :, :],
                    tp[:, 0:len(grp) * 128].rearrange(
                        "p (t ci co) -> p t ci co", t=ntap, co=128))

        # --- load + sign one image into its padded plane --------------------
        # DMA rides the sync ring (explicit bandwidth ordering vs weights);
        # the ACT sign is emitted separately so ACT priority is controlled.
        def load(b):
            tiles = []
            for s in range(CIH):
                xs = xin.tile([128, H * W], F32, tag="xsb", name="xsb")
                nc.sync.dma_start(
                    out=xs, in_=x_d.ap()[b, s * 128:(s + 1) * 128].rearrange(
                        "p a b -> p (a b)"))
                tiles.append(xs)
            return tiles

        def sign(b, tiles):
            xp = xpad[b % 2]
            for s in range(CIH):
                dst = xp[:, s, GO:GO + PLANE].rearrange(
                    "p (y x) -> p y x", x=PW)[:, 1:57, 1:57]
                nc.scalar.sign(dst, tiles[s].rearrange("p (y x) -> p y x", x=W))

        # --- conv for one (image, co-half) ---------------------------------
        def conv(b, h, cpsum):
            xp = xpad[b % 2]
            osb = outp.tile([128, H * W], F32, tag="osb", name="osb")
            c0 = 0
            for t, nch in enumerate(PT_CHUNKS):
                ps = cpsum.tile([128, nch * 512], F32, tag=f"ps{t}", bufs=1,
                                name=f"ps{t}")
                for tap in range(TAPS):
                    dy, dx = tap // KS - 1, tap % KS - 1
                    lhsT = wT[:, tap, h, :, :]
                    for j in range(nch):
                        c = c0 + j
                        off = GO + (1 + RPC * c + dy) * PW + dx
                        o = ps[:, j * 512:j * 512 + CHUNK]
                        if USE_DR:
                            nc.tensor.matmul(
                                o, lhsT, xp[:, :, off:off + CHUNK],
                                start=(tap == 0), stop=(tap == TAPS - 1),
                                perf_mode=mybir.MatmulPerfMode.DoubleRow)
                        else:
                            for s in range(CIH):
                                nc.tensor.matmul(
                                    o, wT[:, tap, h, s, :],
                                    xp[:, s, off:off + CHUNK],
                                    start=(tap == 0 and s == 0),
                                    stop=(tap == TAPS - 1 and s == CIH - 1))
                src = ps.rearrange("p (c e) -> p c e", e=512)[
                    :, 0:nch, 0:CHUNK].rearrange(
                    "p c (r x) -> p c r x", x=PW)[:, :, :, 1:57]
                dst = osb.rearrange("p (y x) -> p y x", x=W)[
                    :, c0 * RPC:(c0 + nch) * RPC, :].rearrange(
                    "p (c r) x -> p c r x", r=RPC)
                # balance PSUM evacuation: co-half 0 on DVE, co-half 1 on ACT
                if h == 0:
                    nc.vector.tensor_scalar(dst, src, scale_alpha[h], None,
                                            mybir.AluOpType.mult)
                else:
                    nc.scalar.activation(dst, src,
                                         mybir.ActivationFunctionType.Copy,
                                         bias=0.0, scale=scale_alpha[h])
                # per-ptile output DMA on the ACT ring (keeps the sync ring
                # free for input DMAs and avoids head-of-line blocking)
                nc.scalar.dma_start(
                    out=o_d.ap()[b, h * 128:(h + 1) * 128,
                                 c0 * RPC:(c0 + nch) * RPC, :].rearrange(
                        "p a b -> p (a b)"),
                    in_=osb[:, c0 * RPC * W:(c0 + nch) * RPC * W])
                c0 += nch

        # --- schedule --------------------------------------------------------
        # x DMAs ride the ACT ring (issue immediately); sign-x sits after
        # sign-w in program order so the weight chain keeps ACT priority.
        with tc.tile_pool(name="cpsum", bufs=1, space="PSUM") as cpsum:
            ws0, wm0 = prep_half(0)
            xt0 = load(0)              # sync-ring order: w-h0, x0, w-h1, x1..
            transpose_half(0, ws0, cpsum)
            reduce_half(0, wm0)
            ws1, wm1 = prep_half(1)
            sign(0, xt0)
            conv(0, 0, cpsum)
            transpose_half(1, ws1, cpsum)
            reduce_half(1, wm1)
            xt1 = load(1)
            sign(1, xt1)
            conv(0, 1, cpsum)
            for b in range(1, BL):
                if b + 1 < BL:
                    xt = load(b + 1)   # prefetch ahead of this image's evacs
                    sign(b + 1, xt)
                conv(b, 0, cpsum)
                conv(b, 1, cpsum)
    nc.compile()
    return nc


def _get_nc():
    if "nc" not in _cache:
        _cache["nc"] = _build()
    return _cache["nc"]


def run(inputs, trace=False):
    nc = _get_nc()
    x = np.ascontiguousarray(inputs["x"], dtype=np.float32)
    in_maps = [
        {
            "x": x[c * BL:(c + 1) * BL],
            "weights": np.ascontiguousarray(inputs["weights"], np.float32),
            "RV": np.ascontiguousarray(inputs["RV"], np.float32),
            "alpha": np.ascontiguousarray(inputs["alpha"], np.float32),
        }
        for c in range(NCORES)
    ]
    res = run_bass_kernel_spmd(nc, in_maps, core_ids=list(range(NCORES)),
                               trace=trace)
    out = np.concatenate([r["out"] for r in res.results], axis=0)
    return out, res


def kernel(**inputs) -> np.ndarray:
    out, _ = run(inputs, trace=False)
    return out


# revision 30
# speedup vs baseline: 1.0432x; 1.0432x over previous
"""Binarized conv2d kernel for Trainium2, SPMD over 8 NeuronCores.

Math (forward-value equivalent of the reference):
    real_w  = sum_k RV[k] * weights[k]          # [256,256,3,3], exact fp32 on DVE
    scale   = mean(|real_w|, axis=(1,2,3))      # per out-channel
    out     = conv2d(sign(x), sign(real_w), pad=1) * (scale * alpha)

sign(x) and sign(real_w) are {-1,0,+1} which are exact in fp8e4, so the conv
is computed with fp8 DoubleRow matmuls (exact integer accumulation in fp32
PSUM) and the per-channel scale*alpha is applied on PSUM evacuation.

Sharding: data-parallel over batch, 4 images per core; weights/RV/alpha
replicated. No collectives.
"""

import numpy as np
from contextlib import ExitStack

import concourse.bass as bass
import concourse.bacc as bacc
import concourse.tile as tile
from concourse import mybir
from concourse.bass_utils import run_bass_kernel_spmd
from concourse.masks import make_identity

# Problem shapes (hardcoded per contract)
B, C, H, W = 32, 256, 56, 56
K, KS = 4, 3
NCORES = 8
BL = B // NCORES            # images per core

PW = W + 2                  # padded width 58
PLANE = PW * PW             # 3364
PL = 3376                   # plane stride (>= 1+PLANE+1, multiple of 16)
GO = 1                      # guard offset: plane data starts at elem 1
RPC = 8                     # rows per chunk
CHUNK = RPC * PW            # 464 elems per matmul (one PSUM bank)
NCHUNK = H // RPC           # 7 chunks: psum tile A gets 4, tile B gets 3
PT_CHUNKS = (4, 3)
CIH = C // 128              # 2 ci halves
COH = C // 128              # 2 co halves
TAPS = KS * KS              # 9

F32 = mybir.dt.float32
FP8 = mybir.dt.float8e4
BF16 = mybir.dt.bfloat16

USE_DR = True               # fp8 DoubleRow (2x matmul) vs bf16

_cache = {}


def _build():
    act_dt = FP8 if USE_DR else BF16
    nc = bacc.Bacc("TRN2", target_bir_lowering=False, debug=False,
                   num_devices=NCORES)
    x_d = nc.dram_tensor("x", [BL, C, H, W], F32, kind="ExternalInput")
    w_d = nc.dram_tensor("weights", [K, C, C, KS, KS], F32, kind="ExternalInput")
    rv_d = nc.dram_tensor("RV", [K + 1], F32, kind="ExternalInput")
    al_d = nc.dram_tensor("alpha", [C, 1, 1], F32, kind="ExternalInput")
    o_d = nc.dram_tensor("out", [BL, C, H, W], F32, kind="ExternalOutput")

    with tile.TileContext(nc) as tc, ExitStack() as ctx:
        consts = ctx.enter_context(tc.tile_pool(name="consts", bufs=1))
        wstage = ctx.enter_context(tc.tile_pool(name="wstage", bufs=16))
        wwork = ctx.enter_context(tc.tile_pool(name="wwork", bufs=2))
        xin = ctx.enter_context(tc.tile_pool(name="xin", bufs=2))
        xpads = ctx.enter_context(tc.tile_pool(name="xpads", bufs=1))
        outp = ctx.enter_context(tc.tile_pool(name="outp", bufs=2))

        # --- tiny constant loads on the ACT HWDGE ring (keeps the sync
        # ring free for the big weight DMAs) -------------------------------
        rv = consts.tile([128, K], F32, tag="rv")
        rv_src = bass.AP(tensor=rv_d.ap().tensor, offset=0,
                         ap=[[0, 128], [1, K]])
        nc.scalar.dma_start(out=rv, in_=rv_src)
        alpha_sb = []
        for h in range(COH):
            t = consts.tile([128, 1], F32, tag=f"alpha{h}")
            nc.scalar.dma_start(out=t,
                                in_=al_d.ap()[h * 128:(h + 1) * 128, 0, :])
            alpha_sb.append(t)

        # Padded planes: zero only the pad borders on DVE (tiny strided
        # memsets — the interior is fully overwritten by sign(x) each image
        # and pads are never written again). GpSimd keeps only the identity.
        xpad = []
        for i in range(2):
            t = xpads.tile([128, CIH, PL], act_dt, tag=f"xpad{i}",
                           name=f"xpad{i}")
            for s in range(CIH):
                pl = t[:, s, :]
                # guard + top row + (1,0)
                nc.vector.memset(pl[:, 0:GO + PW + 1], 0.0)
                # (y,0) and (y,57) for y=1..56
                nc.vector.memset(
                    pl[:, GO + PW:GO + PW + H * PW].rearrange(
                        "p (r c) -> p r c", c=PW)[:, :, 0:1], 0.0)
                nc.vector.memset(
                    pl[:, GO + PW + PW - 1:GO + PW + PW - 1 + H * PW].rearrange(
                        "p (r c) -> p r c", c=PW)[:, :, 0:1], 0.0)
                # bottom row + trailing guard/pad
                nc.vector.memset(pl[:, GO + (PW - 1) * PW:PL], 0.0)
            xpad.append(t)
        ident = consts.tile([128, 128], act_dt, tag="ident")
        make_identity(nc, ident)

        wT = consts.tile([128, TAPS, COH, CIH, 128], act_dt, tag="wT")
        scale_alpha = [consts.tile([128, 1], F32, tag=f"sa{h}", name=f"sa{h}")
                       for h in range(COH)]

        # --- weight prep for one co-half: DMA, mix, scale, sign ------------
        # ci-split (HCI columns at a time) so the mix/sign tail trails the
        # weight DMA by one sub-pass instead of the whole 4.7MB.
        HCI = C // CIH * TAPS  # 1152 columns per ci-half
        def prep_half(h):
            # (TensorScalarPtr is DVE-only in walrus codegen — Pool rejects)
            mixeng = nc.vector
            wmix = wwork.tile([128, C * TAPS], F32, tag="wmix", name="wmix")
            ws = wwork.tile([128, C * TAPS], act_dt, tag=f"wsign{h}", bufs=1,
                            name=f"wsign{h}")
            for ci in range(CIH):
                for k in range(K):
                    wk = wstage.tile([128, HCI], F32, tag="wsb", name="wk")
                    nc.sync.dma_start(
                        out=wk,
                        in_=w_d.ap()[k, h * 128:(h + 1) * 128,
                                     ci * (C // CIH):(ci + 1) * (C // CIH)]
                        .rearrange("p c a b -> p (c a b)"))
                    dst = wmix[:, ci * HCI:(ci + 1) * HCI]
                    mixeng.scalar_tensor_tensor(
                        dst, wk, rv[:, k:k + 1], wk if k == 0 else dst,
                        mybir.AluOpType.mult,
                        mybir.AluOpType.bypass if k == 0 else
                        mybir.AluOpType.add)
                nc.scalar.sign(ws[:, ci * HCI:(ci + 1) * HCI],
                               wmix[:, ci * HCI:(ci + 1) * HCI])
            return ws, wmix

        # |real_w| row-sums + scale*alpha combine, on DVE (fills DMA-gated
        # bubbles between mix passes)
        def reduce_half(h, wmix):
            absum = consts.tile([128, 1], F32, tag=f"ab{h}", name=f"ab{h}")
            nc.vector.tensor_reduce(absum, wmix, mybir.AxisListType.X,
                                    mybir.AluOpType.add,
                                    apply_absolute_value=True)
            nc.vector.scalar_tensor_tensor(
                scale_alpha[h], absum, 1.0 / (C * TAPS), alpha_sb[h],
                mybir.AluOpType.mult, mybir.AluOpType.mult)

        # --- transpose one co-half's sign-weights into wT -------------------
        # Groups of 4 [128,128] transposes fill the dedicated 1-bank psum
        # tile (tag tps); each group is evacuated with one DVE copy.
        def transpose_half(h, wsgn, cpsum):
            wsv = wsgn.rearrange("p (ci t) -> p ci t", t=TAPS)
            pairs = [(t, ci) for t in range(TAPS) for ci in range(CIH)]
            for g0 in range(0, len(pairs), 4):
                grp = pairs[g0:g0 + 4]
                tp = cpsum.tile([128, 512], F32, tag="tps", bufs=1, name="tp")
                for i, (tap, ci) in enumerate(grp):
                    nc.tensor.matmul(
                        tp[:, i * 128:(i + 1) * 128],
                        wsv[:, ci * 128:(ci + 1) * 128, tap], ident,
                        start=True, stop=True)
                t0, ci0 = grp[0]
                ntap = len(grp) // CIH
                nc.scalar.copy(
                    wT[:, t0:t0 + ntap, h, :, :],
                    tp[:, 0:len(grp) * 128].rearrange(
                        "p (t ci co) -> p t ci co", t=ntap, co=128))

        # --- load + sign one image into its padded plane --------------------
        # DMA rides the sync ring (explicit bandwidth ordering vs weights);
        # the ACT sign is emitted separately so ACT priority is controlled.
        def load(b):
            tiles = []
            for s in range(CIH):
                xs = xin.tile([128, H * W], F32, tag="xsb", name="xsb")
                nc.sync.dma_start(
                    out=xs, in_=x_d.ap()[b, s * 128:(s + 1) * 128].rearrange(
                        "p a b -> p (a b)"))
                tiles.append(xs)
            return tiles

        def sign(b, tiles):
            xp = xpad[b % 2]
            for s in range(CIH):
                dst = xp[:, s, GO:GO + PLANE].rearrange(
                    "p (y x) -> p y x", x=PW)[:, 1:57, 1:57]
                nc.scalar.sign(dst, tiles[s].rearrange("p (y x) -> p y x", x=W))

        # --- conv for one (image, co-half) ---------------------------------
        def conv(b, h, cpsum):
            xp = xpad[b % 2]
            osb = outp.tile([128, H * W], F32, tag="osb", name="osb")
            c0 = 0
            for t, nch in enumerate(PT_CHUNKS):
                ps = cpsum.tile([128, nch * 512], F32, tag=f"ps{t}", bufs=1,
                                name=f"ps{t}")
                for tap in range(TAPS):
                    dy, dx = tap // KS - 1, tap % KS - 1
                    lhsT = wT[:, tap, h, :, :]
                    for j in range(nch):
                        c = c0 + j
                        off = GO + (1 + RPC * c + dy) * PW + dx
                        o = ps[:, j * 512:j * 512 + CHUNK]
                        if USE_DR:
                            nc.tensor.matmul(
                                o, lhsT, xp[:, :, off:off + CHUNK],
                                start=(tap == 0), stop=(tap == TAPS - 1),
                                perf_mode=mybir.MatmulPerfMode.DoubleRow)
                        else:
                            for s in range(CIH):
                                nc.tensor.matmul(
                                    o, wT[:, tap, h, s, :],
                                    xp[:, s, off:off + CHUNK],
                                    start=(tap == 0 and s == 0),
                                    stop=(tap == TAPS - 1 and s == CIH - 1))
                src = ps.rearrange("p (c e) -> p c e", e=512)[
                    :, 0:nch, 0:CHUNK].rearrange(
                    "p c (r x) -> p c r x", x=PW)[:, :, :, 1:57]
                dst = osb.rearrange("p (y x) -> p y x", x=W)[
                    :, c0 * RPC:(c0 + nch) * RPC, :].rearrange(
                    "p (c r) x -> p c r x", r=RPC)
                # all PSUM evacuation on ACT: DVE must stay clear for the
                # weight mix during startup (static DVE ordering would
                # otherwise put evacs ahead of the mix tail)
                nc.scalar.activation(dst, src,
                                     mybir.ActivationFunctionType.Copy,
                                     bias=0.0, scale=scale_alpha[h])
                # per-ptile output DMA on the ACT ring (keeps the sync ring
                # free for input DMAs and avoids head-of-line blocking)
                nc.scalar.dma_start(
                    out=o_d.ap()[b, h * 128:(h + 1) * 128,
                                 c0 * RPC:(c0 + nch) * RPC, :].rearrange(
                        "p a b -> p (a b)"),
                    in_=osb[:, c0 * RPC * W:(c0 + nch) * RPC * W])
                c0 += nch

        # --- schedule --------------------------------------------------------
        # x DMAs ride the ACT ring (issue immediately); sign-x sits after
        # sign-w in program order so the weight chain keeps ACT priority.
        with tc.tile_pool(name="cpsum", bufs=1, space="PSUM") as cpsum:
            xt0 = load(0)              # sync-ring order: x0, w-h0, w-h1, x1..
            sign(0, xt0)
            ws0, wm0 = prep_half(0)
            transpose_half(0, ws0, cpsum)
            reduce_half(0, wm0)
            ws1, wm1 = prep_half(1)
            conv(0, 0, cpsum)
            transpose_half(1, ws1, cpsum)
            reduce_half(1, wm1)
            xt1 = load(1)
            sign(1, xt1)
            conv(0, 1, cpsum)
            for b in range(1, BL):
                if b + 1 < BL:
                    xt = load(b + 1)   # prefetch ahead of this image's evacs
                    sign(b + 1, xt)
                conv(b, 0, cpsum)
                conv(b, 1, cpsum)
    nc.compile()
    return nc


def _get_nc():
    if "nc" not in _cache:
        _cache["nc"] = _build()
    return _cache["nc"]


def run(inputs, trace=False):
    nc = _get_nc()
    x = np.ascontiguousarray(inputs["x"], dtype=np.float32)
    in_maps = [
        {
            "x": x[c * BL:(c + 1) * BL],
            "weights": np.ascontiguousarray(inputs["weights"], np.float32),
            "RV": np.ascontiguousarray(inputs["RV"], np.float32),
            "alpha": np.ascontiguousarray(inputs["alpha"], np.float32),
        }
        for c in range(NCORES)
    ]
    res = run_bass_kernel_spmd(nc, in_maps, core_ids=list(range(NCORES)),
                               trace=trace)
    out = np.concatenate([r["out"] for r in res.results], axis=0)
    return out, res


def kernel(**inputs) -> np.ndarray:
    out, _ = run(inputs, trace=False)
    return out
